# revision 12
# baseline (speedup 1.0000x reference)
"""CrossModalAttention kernel for 8 trn2 NeuronCores.

Sharding: data-parallel over (batch, query-slice): core i handles batch i//4,
query rows (i%4)*512..+512. Each core computes K/V projections for its batch
redundantly (avoids all cross-core communication), attention for all 16 heads
over its 512 queries, output projection + residual + LayerNorm.

Per-core pipeline (all matmuls bf16, accumulation fp32):
  1. PE-transpose key/value/query slices to c-major (bf16)
  2. K^T[hd,kv], V[kv,hd] (+ ones column for softmax denominators), Q^T[hd,q]
  3. per head: scores^T[kv,q] -> exp (ACT, temp applied via per-partition
     scale AP) -> context^T = V^T.exp accumulation; row 64 of the V_aug
     matmul gives the softmax denominator for free
  4. denominators -> reciprocal -> DRAM roundtrip to transpose into
     q-partition-major; ln() gives the bias that fuses softmax
     normalization into the q-major exp
  5. per head q-major scores -> exp(temp*s + ln(1/denom)) = normalized
     attention weights -> DMA out
  6. out-projection from context^T (c-major) x Wo^T + bias + residual + LN
"""
import numpy as np
import ml_dtypes

import concourse.bass as bass
import concourse.bacc as bacc
import concourse.mybir as mybir
import concourse.tile as tile
from concourse.bass_utils import run_bass_kernel_spmd
from concourse.masks import make_identity

P = 128
H = 1024          # hidden
NH = 16           # heads
DH = 64           # head dim
SQ = 512          # per-core query rows
SKV = 2048        # kv length
B = 2
SQ_FULL = 2048
CT = H // P       # 8 c-tiles
KVT = SKV // P    # 16 kv-tiles
QTT = SQ // P     # 4 q-tiles
F32 = mybir.dt.float32
BF16 = mybir.dt.bfloat16
AF = mybir.ActivationFunctionType
ALU = mybir.AluOpType


def _build():
    nc = bacc.Bacc("TRN2", target_bir_lowering=False, debug=False, num_devices=8)

    qs = nc.dram_tensor("qs", [SQ, H], F32, kind="ExternalInput")
    kb = nc.dram_tensor("kb", [SKV, H], F32, kind="ExternalInput")
    vb = nc.dram_tensor("vb", [SKV, H], F32, kind="ExternalInput")
    wqt = nc.dram_tensor("wqt", [H, H], BF16, kind="ExternalInput")
    wkt = nc.dram_tensor("wkt", [H, H], BF16, kind="ExternalInput")
    wvt = nc.dram_tensor("wvt", [H, H], BF16, kind="ExternalInput")
    wot = nc.dram_tensor("wot", [H, H], BF16, kind="ExternalInput")
    bq = nc.dram_tensor("bq", [H], F32, kind="ExternalInput")
    bk = nc.dram_tensor("bk", [H], F32, kind="ExternalInput")
    bv = nc.dram_tensor("bv", [H], F32, kind="ExternalInput")
    bo = nc.dram_tensor("bo", [H], F32, kind="ExternalInput")
    gamma = nc.dram_tensor("gamma", [H], F32, kind="ExternalInput")
    beta = nc.dram_tensor("beta", [H], F32, kind="ExternalInput")
    temp = nc.dram_tensor("temp", [1], F32, kind="ExternalInput")
    attn_o = nc.dram_tensor("attn_o", [NH, SQ, SKV], F32, kind="ExternalOutput")
    out_o = nc.dram_tensor("out_o", [SQ, H], F32, kind="ExternalOutput")

    with tile.TileContext(nc) as tc:
        with (
            tc.tile_pool(name="single", bufs=1) as single,
            tc.tile_pool(name="ld", bufs=1) as ld,
            tc.tile_pool(name="cb", bufs=2) as cb,
            tc.tile_pool(name="wt", bufs=1) as wt,
            tc.tile_pool(name="ex", bufs=2) as ex,
            tc.tile_pool(name="at", bufs=2) as at,
            tc.tile_pool(name="rbp", bufs=1) as rbp,
            tc.tile_pool(name="op", bufs=2) as op,
            tc.tile_pool(name="lnp", bufs=2) as lnp,
            tc.tile_pool(name="dram", bufs=1, space="DRAM") as dram,
            tc.tile_pool(name="ps", bufs=4, space="PSUM") as ps,
        ):
            # ---- constants ----
            ident = single.tile([P, P], BF16)
            make_identity(nc, ident)
            temp_b = single.tile([P, 1], F32)
            nc.sync.dma_start(out=temp_b, in_=bass.AP(tensor=temp, offset=0, ap=[[0, P], [1, 1]]))
            bq_sb = single.tile([P, CT], F32)
            nc.sync.dma_start(out=bq_sb, in_=bq.rearrange("(o p) -> p o", p=P))
            bk_sb = single.tile([P, CT], F32)
            nc.sync.dma_start(out=bk_sb, in_=bk.rearrange("(o p) -> p o", p=P))
            bv_b = single.tile([P, H], F32)
            nc.sync.dma_start(out=bv_b, in_=bass.AP(tensor=bv, offset=0, ap=[[0, P], [1, H]]))
            bo_b = single.tile([P, H], F32)
            nc.sync.dma_start(out=bo_b, in_=bass.AP(tensor=bo, offset=0, ap=[[0, P], [1, H]]))
            gam_b = single.tile([P, H], F32)
            nc.sync.dma_start(out=gam_b, in_=bass.AP(tensor=gamma, offset=0, ap=[[0, P], [1, H]]))
            bet_b = single.tile([P, H], F32)
            nc.sync.dma_start(out=bet_b, in_=bass.AP(tensor=beta, offset=0, ap=[[0, P], [1, H]]))
            eps_sb = single.tile([P, 1], F32)
            nc.vector.memset(eps_sb, 1e-5)

            # prime the exp table load before the hot loops
            prime1 = single.tile([P, 1], F32)
            nc.scalar.activation(out=prime1, in_=temp_b, func=AF.Exp)

            # ---- persistent tensors ----
            xT = single.tile([P, CT, SKV], BF16)      # transposed activations (c-major)
            KT = single.tile([P, CT, SKV], BF16)      # K^T  [hd, kv]
            Vaug = single.tile([P, KVT, NH * 65], BF16)  # V[kv, h*65+d], col 64 = ones
            QT = single.tile([P, CT, SQ], BF16)       # Q^T  [hd, q]
            ctxT = single.tile([P, CT, SQ], BF16)     # context^T [hd, q] (normalized)
            recip_all = single.tile([1, NH * SQ], F32)
            lnr_in = single.tile([P, NH * QTT], F32)
            lnr = single.tile([P, NH * QTT], F32)
            recip_dram = dram.tile([NH * SQ], F32)

            def transpose_in(src_dram, rows, col_limit):
                # src [rows, H] f32 -> xT[:, ct, 0:rows] bf16 (c-major)
                for st in range(rows // P):
                    ldt = ld.tile([P, H], F32, tag="xf")
                    nc.sync.dma_start(out=ldt, in_=src_dram[st * P:(st + 1) * P, :])
                    cst = cb.tile([P, H], BF16, tag="xb")
                    nc.vector.tensor_copy(out=cst, in_=ldt)
                    for ct in range(CT):
                        pt = ps.tile([P, P], BF16, tag="ps")
                        nc.tensor.transpose(pt, cst[:, ct * P:(ct + 1) * P], ident)
                        nc.any.tensor_copy(out=xT[:, ct, st * P:(st + 1) * P], in_=pt)

            # ---- K^T ----
            transpose_in(kb, SKV, SKV)
            wk_sb = wt.tile([P, CT, H], BF16, tag="w")
            nc.sync.dma_start(out=wk_sb, in_=wkt.rearrange("(o p) e -> p o e", p=P))
            for pt_i in range(CT):
                for nt in range(4):
                    psm = ps.tile([P, 512], F32, tag="ps")
                    for ct in range(CT):
                        nc.tensor.matmul(psm, wk_sb[:, ct, pt_i * P:(pt_i + 1) * P],
                                         xT[:, ct, nt * 512:(nt + 1) * 512],
                                         start=(ct == 0), stop=(ct == CT - 1))
                    nc.vector.tensor_scalar(out=KT[:, pt_i, nt * 512:(nt + 1) * 512],
                                            in0=psm, scalar1=bk_sb[:, pt_i:pt_i + 1],
                                            scalar2=None, op0=ALU.add)

            # ---- V (augmented with ones column) ----
            transpose_in(vb, SKV, SKV)
            wv_sb = wt.tile([P, CT, H], BF16, tag="w")
            nc.sync.dma_start(out=wv_sb, in_=wvt.rearrange("(o p) e -> p o e", p=P))
            for st in range(KVT):
                vrow = Vaug[:, st, :].rearrange("p (h d) -> p h d", d=65)
                for nt in range(2):
                    psm = ps.tile([P, 512], F32, tag="ps")
                    for ct in range(CT):
                        nc.tensor.matmul(psm, xT[:, ct, st * P:(st + 1) * P],
                                         wv_sb[:, ct, nt * 512:(nt + 1) * 512],
                                         start=(ct == 0), stop=(ct == CT - 1))
                    nc.vector.tensor_tensor(
                        out=vrow[:, nt * 8:(nt + 1) * 8, 0:64],
                        in0=psm.rearrange("p (h d) -> p h d", d=64),
                        in1=bv_b[:, nt * 512:(nt + 1) * 512].rearrange("p (h d) -> p h d", d=64),
                        op=ALU.add)
                nc.vector.memset(vrow[:, :, 64:65], 1.0)

            # ---- Q^T ----
            transpose_in(qs, SQ, SQ)
            wq_sb = wt.tile([P, CT, H], BF16, tag="w")
            nc.sync.dma_start(out=wq_sb, in_=wqt.rearrange("(o p) e -> p o e", p=P))
            for pt_i in range(CT):
                psm = ps.tile([P, 512], F32, tag="ps")
                for ct in range(CT):
                    nc.tensor.matmul(psm, wq_sb[:, ct, pt_i * P:(pt_i + 1) * P],
                                     xT[:, ct, 0:SQ],
                                     start=(ct == 0), stop=(ct == CT - 1))
                nc.vector.tensor_scalar(out=QT[:, pt_i, :], in0=psm,
                                        scalar1=bq_sb[:, pt_i:pt_i + 1],
                                        scalar2=None, op0=ALU.add)

            # ---- phase A (head pairs): kv-major scores -> exp -> context^T ----
            # Even/odd heads of a pair live on partition rows 0-63 / 64-127 of
            # the same ptile, so their K=64 matmuls are issued adjacently and
            # run concurrently on disjoint PE row groups.
            for hp in range(NH // 2):
                pt_h = hp
                ctx_e = ps.tile([P, 512], F32, tag="ps")
                ctx_o = ps.tile([P, 512], F32, tag="ps")
                for g in range(8):
                    sc_e = ps.tile([P, 1024], F32, tag="ps")
                    sc_o = ps.tile([P, 1024], F32, tag="ps")
                    for j in range(2):
                        kvt = g * 2 + j
                        nc.tensor.matmul(sc_e[:, j * 512:(j + 1) * 512],
                                         KT[0:DH, pt_h, kvt * P:(kvt + 1) * P],
                                         QT[0:DH, pt_h, :], start=True, stop=True)
                        nc.tensor.matmul(sc_o[:, j * 512:(j + 1) * 512],
                                         KT[DH:P, pt_h, kvt * P:(kvt + 1) * P],
                                         QT[DH:P, pt_h, :], start=True, stop=True)
                    exT_e = ex.tile([P, 1024], BF16, tag="ex")
                    nc.scalar.activation(out=exT_e, in_=sc_e, func=AF.Exp, scale=temp_b)
                    exT_o = ex.tile([P, 1024], BF16, tag="ex")
                    nc.scalar.activation(out=exT_o, in_=sc_o, func=AF.Exp, scale=temp_b)
                    for j in range(2):
                        kvt = g * 2 + j
                        nc.tensor.matmul(ctx_e[0:65, :],
                                         Vaug[:, kvt, (2 * hp) * 65:(2 * hp + 1) * 65],
                                         exT_e[:, j * 512:(j + 1) * 512],
                                         start=(kvt == 0), stop=(kvt == KVT - 1))
                        nc.tensor.matmul(ctx_o[0:65, :],
                                         Vaug[:, kvt, (2 * hp + 1) * 65:(2 * hp + 2) * 65],
                                         exT_o[:, j * 512:(j + 1) * 512],
                                         start=(kvt == 0), stop=(kvt == KVT - 1))
                for h, ctx_ps, o0 in ((2 * hp, ctx_e, 0), (2 * hp + 1, ctx_o, DH)):
                    # free ctx psum fast: copy unnormalized, normalize in place later
                    nc.vector.reciprocal(out=recip_all[0:1, h * SQ:(h + 1) * SQ],
                                         in_=ctx_ps[64:65, :])
                    nc.vector.tensor_copy(out=ctxT[o0:o0 + DH, pt_h, :],
                                          in_=ctx_ps[0:64, :])
                    nc.gpsimd.dma_start(out=recip_dram[h * SQ:(h + 1) * SQ],
                                        in_=recip_all[0:1, h * SQ:(h + 1) * SQ])
                    rb = rbp.tile([P, 512], F32, tag="rb")
                    nc.gpsimd.dma_start(out=rb,
                                        in_=bass.AP(tensor=recip_dram.tensor,
                                                    offset=recip_dram.offset + h * SQ,
                                                    ap=[[0, P], [1, 512]]))
                    nc.vector.tensor_tensor(out=ctxT[o0:o0 + DH, pt_h, :],
                                            in0=ctxT[o0:o0 + DH, pt_h, :],
                                            in1=rb[o0:o0 + DH, :],
                                            op=ALU.mult)

            # ---- ln(1/denom) in q-partition-major, one gather + one Ln ----
            nc.gpsimd.dma_start(out=lnr_in,
                                in_=bass.AP(tensor=recip_dram.tensor,
                                            offset=recip_dram.offset,
                                            ap=[[1, P], [SQ, NH], [P, QTT]]))
            nc.scalar.activation(out=lnr, in_=lnr_in, func=AF.Ln)

            # ---- phase C (head pairs): q-major normalized attention weights ----
            for hp in range(NH // 2):
                pt_h = hp
                for qt in range(QTT):
                    for kc in range(2):
                        q_e = ps.tile([P, 1024], F32, tag="ps")
                        q_o = ps.tile([P, 1024], F32, tag="ps")
                        for j in range(2):
                            k0 = kc * 1024 + j * 512
                            nc.tensor.matmul(q_e[:, j * 512:(j + 1) * 512],
                                             QT[0:DH, pt_h, qt * P:(qt + 1) * P],
                                             KT[0:DH, pt_h, k0:k0 + 512],
                                             start=True, stop=True)
                            nc.tensor.matmul(q_o[:, j * 512:(j + 1) * 512],
                                             QT[DH:P, pt_h, qt * P:(qt + 1) * P],
                                             KT[DH:P, pt_h, k0:k0 + 512],
                                             start=True, stop=True)
                        for h, q_ps in ((2 * hp, q_e), (2 * hp + 1, q_o)):
                            a_sb = at.tile([P, 1024], F32, tag="at")
                            nc.scalar.activation(out=a_sb, in_=q_ps, func=AF.Exp,
                                                 scale=temp_b,
                                                 bias=lnr[:, h * QTT + qt:h * QTT + qt + 1])
                            nc.sync.dma_start(
                                out=attn_o[h, qt * P:(qt + 1) * P,
                                           kc * 1024:(kc + 1) * 1024],
                                in_=a_sb)

            # ---- phase D: out-projection + residual + LayerNorm ----
            wo_sb = wt.tile([P, CT, H], BF16, tag="w")
            nc.sync.dma_start(out=wo_sb, in_=wot.rearrange("(o p) e -> p o e", p=P))
            for qt in range(QTT):
                o_sb = op.tile([P, H], F32, tag="o")
                for half in range(2):
                    o_ps = ps.tile([P, 512], F32, tag="ps")
                    for ct in range(CT):
                        nc.tensor.matmul(o_ps, ctxT[:, ct, qt * P:(qt + 1) * P],
                                         wo_sb[:, ct, half * 512:(half + 1) * 512],
                                         start=(ct == 0), stop=(ct == CT - 1))
                    nc.vector.tensor_tensor(out=o_sb[:, half * 512:(half + 1) * 512],
                                            in0=o_ps, in1=bo_b[:, half * 512:(half + 1) * 512],
                                            op=ALU.add)
                qres = ld.tile([P, H], F32, tag="xf")
                nc.sync.dma_start(out=qres, in_=qs[qt * P:(qt + 1) * P, :])
                nc.vector.tensor_tensor(out=o_sb, in0=o_sb, in1=qres, op=ALU.add)
                stats = lnp.tile([P, 2, 6], F32, tag="st")
                nc.vector.bn_stats(out=stats[:, 0, :], in_=o_sb[:, 0:512])
                nc.vector.bn_stats(out=stats[:, 1, :], in_=o_sb[:, 512:1024])
                mv = lnp.tile([P, 2], F32, tag="mv")
                nc.vector.bn_aggr(out=mv, in_=stats)
                rstd = lnp.tile([P, 1], F32, tag="rs")
                nc.scalar.activation(out=rstd, in_=mv[:, 1:2], func=AF.Sqrt, bias=eps_sb)
                nc.vector.reciprocal(out=rstd, in_=rstd)
                nc.vector.tensor_scalar(out=o_sb, in0=o_sb, scalar1=mv[:, 0:1],
                                        scalar2=rstd, op0=ALU.subtract, op1=ALU.mult)
                nc.vector.tensor_tensor(out=o_sb, in0=o_sb, in1=gam_b, op=ALU.mult)
                nc.vector.tensor_tensor(out=o_sb, in0=o_sb, in1=bet_b, op=ALU.add)
                nc.sync.dma_start(out=out_o[qt * P:(qt + 1) * P, :], in_=o_sb)

    nc.finalize()
    return nc


_NC_CACHE = {}


def kernel(query, key, value, Wq, bq, Wk, bk, Wv, bv, Wo, bo, ln_gamma, ln_beta,
           temperature, _trace=False):
    query = np.asarray(query, np.float32)
    key = np.asarray(key, np.float32)
    value = np.asarray(value, np.float32)
    wqt = np.ascontiguousarray(np.asarray(Wq, np.float32).T).astype(ml_dtypes.bfloat16)
    wkt = np.ascontiguousarray(np.asarray(Wk, np.float32).T).astype(ml_dtypes.bfloat16)
    wvt = np.ascontiguousarray(np.asarray(Wv, np.float32).T).astype(ml_dtypes.bfloat16)
    wot = np.ascontiguousarray(np.asarray(Wo, np.float32).T).astype(ml_dtypes.bfloat16)
    consts = dict(
        wqt=wqt, wkt=wkt, wvt=wvt, wot=wot,
        bq=np.asarray(bq, np.float32), bk=np.asarray(bk, np.float32),
        bv=np.asarray(bv, np.float32), bo=np.asarray(bo, np.float32),
        gamma=np.asarray(ln_gamma, np.float32), beta=np.asarray(ln_beta, np.float32),
        temp=np.asarray(temperature, np.float32),
    )
    in_maps = []
    for core in range(8):
        b = core // 4
        q0 = (core % 4) * SQ
        in_maps.append(dict(
            qs=np.ascontiguousarray(query[b, q0:q0 + SQ, :]),
            kb=np.ascontiguousarray(key[b]),
            vb=np.ascontiguousarray(value[b]),
            **consts,
        ))

    if "nc" not in _NC_CACHE:
        _NC_CACHE["nc"] = _build()
    nc = _NC_CACHE["nc"]
    res = run_bass_kernel_spmd(nc, in_maps, core_ids=list(range(8)), trace=_trace)

    attn = np.empty((B, NH, SQ_FULL, SKV), np.float32)
    out = np.empty((B, SQ_FULL, H), np.float32)
    for core in range(8):
        b = core // 4
        q0 = (core % 4) * SQ
        attn[b, :, q0:q0 + SQ, :] = res.results[core]["attn_o"]
        out[b, q0:q0 + SQ, :] = res.results[core]["out_o"]
    if _trace:
        kernel._last_trace = res
    return out, attn
